# revision 3
# baseline (speedup 1.0000x reference)
"""Trainium2 Bass kernel for ArcticMLP MoE grouped-GEMM (nn_ArcticMLPMoE).

Reference computation (per token group g of expert e, tokens sorted by expert):
    gate = x @ w1[e];  up = x @ w3[e];  out = (silu(gate) * up) @ w2[e]

Strategy
--------
Expert-parallel across the 8 NeuronCores: tokens arrive pre-sorted by
expert, so each core owns E/8 experts and their token slices -- zero
collectives.  The problem is weight-DMA bound (each weight byte is used
for only 128 tokens), so on the host we:
  * split tokens into 128-token buckets per expert (general ragged
    group_sizes supported via zero-padding; the standard case of 128
    tokens/expert is a pure reshape),
  * downcast weights/activations to bf16 (halves the HBM traffic;
    matmuls accumulate in fp32 PSUM, rel. error ~5e-3 << 2e-2),
  * pre-tile every tensor so each device DMA is fully contiguous.

Per bucket (128 tokens) the device streams w1/w3/w2 in F-chunks of 512:
    gate/up [128t x 512f] = sum_h xT[h,t].T @ w{1,3}[h,f]   (8 k-tiles)
    inter   = silu(gate) * up                  (ACT + DVE, fp32->bf16)
    interT  [f,t] via PE transpose (identity matmul)
    out    += interT.T @ w2[f,h]               (accumulated in PSUM)
"""

import os
import sys

import numpy as np

sys.path.insert(0, "/opt/trn_rl_repo")

E = 32
H = 1024
F = 2048
T = 4096
N_CORES = 8
TOK = 128          # tokens per bucket (= per expert in the standard case)
FC = 512           # F-chunk width (moving-operand free dim for gate/up)
N_FC = F // FC     # 4 chunks
HT = H // 128      # 8 k-tiles over hidden dim
FT = FC // 128     # 4 f-tiles per chunk

_COMPILED = {}     # buckets_per_core -> (nc, param_names)


def _build(nbpc: int):
    """Build + compile the per-core Bass graph for `nbpc` buckets/core."""
    from contextlib import ExitStack

    import concourse.bass as bass
    import concourse.mybir as mybir
    import concourse.tile as tile
    from concourse import bacc
    from concourse.masks import make_identity

    BF16 = mybir.dt.bfloat16
    F32 = mybir.dt.float32
    AF = mybir.ActivationFunctionType
    TPC = nbpc * TOK   # tokens per core

    nc = bacc.Bacc(
        "TRN2", target_bir_lowering=False, debug=False, num_devices=N_CORES
    )

    xT_d = nc.dram_tensor("xt", [128, HT, TPC], BF16, kind="ExternalInput")
    w1_d = nc.dram_tensor("w1", [nbpc, N_FC, 128, HT, FC], BF16, kind="ExternalInput")
    w3_d = nc.dram_tensor("w3", [nbpc, N_FC, 128, HT, FC], BF16, kind="ExternalInput")
    w2_d = nc.dram_tensor("w2", [nbpc, N_FC, 128, FT, H], BF16, kind="ExternalInput")
    out_d = nc.dram_tensor("out", [TPC, H], F32, kind="ExternalOutput")

    with tile.TileContext(nc) as tc, ExitStack() as ctx:
        consts = ctx.enter_context(tc.tile_pool(name="consts", bufs=1))
        xpool = ctx.enter_context(tc.tile_pool(name="xpool", bufs=1))
        wpool = ctx.enter_context(tc.tile_pool(name="wpool", bufs=4))
        epool = ctx.enter_context(tc.tile_pool(name="epool", bufs=2))
        pg = ctx.enter_context(tc.tile_pool(name="pg", bufs=2, space="PSUM"))
        pt = ctx.enter_context(tc.tile_pool(name="pt", bufs=2, space="PSUM"))
        po = ctx.enter_context(tc.tile_pool(name="po", bufs=1, space="PSUM"))

        ident = consts.tile([128, 128], BF16)
        make_identity(nc, ident[:])

        xT = xpool.tile([128, HT, TPC], BF16)
        nc.sync.dma_start(out=xT[:], in_=xT_d[:])

        for b in range(nbpc):
            out_ps = po.tile([128, H], F32, tag="out_ps")
            for fc in range(N_FC):
                w1c = wpool.tile([128, HT, FC], BF16, tag="w1c")
                nc.sync.dma_start(out=w1c[:], in_=w1_d[b, fc])
                w3c = wpool.tile([128, HT, FC], BF16, tag="w3c")
                nc.sync.dma_start(out=w3c[:], in_=w3_d[b, fc])
                w2c = wpool.tile([128, FT, H], BF16, tag="w2c")
                nc.sync.dma_start(out=w2c[:], in_=w2_d[b, fc])

                gate = pg.tile([128, FC], F32, tag="gate")
                up = pg.tile([128, FC], F32, tag="up")
                for a in range(HT):
                    lhs = xT[:, a, b * TOK:(b + 1) * TOK]
                    nc.tensor.matmul(
                        gate[:], lhs, w1c[:, a, :],
                        start=(a == 0), stop=(a == HT - 1),
                    )
                    nc.tensor.matmul(
                        up[:], lhs, w3c[:, a, :],
                        start=(a == 0), stop=(a == HT - 1),
                    )

                silu = epool.tile([128, FC], F32, tag="silu")
                nc.scalar.activation(silu[:], gate[:], AF.Silu)
                inter = epool.tile([128, FC], BF16, tag="inter")
                nc.vector.tensor_mul(inter[:], silu[:], up[:])

                interT = epool.tile([128, FT, TOK], BF16, tag="interT")
                for ft in range(FT):
                    tps = pt.tile([128, TOK], BF16, tag="tps")
                    nc.tensor.transpose(
                        tps[:], inter[:, ft * 128:(ft + 1) * 128], ident[:]
                    )
                    nc.vector.tensor_copy(interT[:, ft, :], tps[:])

                for ft in range(FT):
                    first = fc == 0 and ft == 0
                    last = fc == N_FC - 1 and ft == FT - 1
                    for n in range(2):
                        nc.tensor.matmul(
                            out_ps[:, n * 512:(n + 1) * 512],
                            interT[:, ft, :],
                            w2c[:, ft, n * 512:(n + 1) * 512],
                            start=first, stop=last,
                        )

            outs = epool.tile([128, H], F32, tag="outs")
            nc.vector.tensor_copy(outs[:], out_ps[:])
            nc.sync.dma_start(out=out_d[b * TOK:(b + 1) * TOK, :], in_=outs[:])

    nc.compile()
    return nc


def _get_compiled(nbpc: int):
    if nbpc not in _COMPILED:
        _COMPILED[nbpc] = _build(nbpc)
    return _COMPILED[nbpc]


def _plan_buckets(group_sizes):
    """Split ragged expert groups into <=128-token buckets.

    Returns list of (expert_id, token_start, ntok)."""
    buckets = []
    start = 0
    for e, g in enumerate(np.asarray(group_sizes).astype(np.int64)):
        off = 0
        while off < g:
            n = min(TOK, g - off)
            buckets.append((e, start + off, int(n)))
            off += n
        start += int(g)
    return buckets


def _prepare_in_maps(hidden_states, w1, w3, w2, buckets, nbpc):
    import ml_dtypes

    bf16 = ml_dtypes.bfloat16
    nb = nbpc * N_CORES

    w1b = np.asarray(w1, dtype=bf16)
    w3b = np.asarray(w3, dtype=bf16)
    w2b = np.asarray(w2, dtype=bf16)
    hsb = np.asarray(hidden_states, dtype=bf16)

    # Token buckets: [nb, TOK, H], zero-padded.
    uniform = (
        len(buckets) == nb
        and all(n == TOK for (_, _, n) in buckets)
        and all(s == i * TOK for i, (_, s, _) in enumerate(buckets))
    )
    if uniform:
        xb = hsb.reshape(nb, TOK, H)
        eids = np.array([e for (e, _, _) in buckets])
    else:
        xb = np.zeros((nb, TOK, H), dtype=bf16)
        eids = np.zeros(nb, dtype=np.int64)
        for i, (e, s, n) in enumerate(buckets):
            xb[i, :n] = hsb[s:s + n]
            eids[i] = e

    # Per-bucket weights (gather; identity when one bucket per expert).
    w1g = w1b[eids]  # [nb, H, F]
    w3g = w3b[eids]
    w2g = w2b[eids]  # [nb, F, H]

    # Device layouts (everything contiguous per DMA):
    #  xT  [128p(h%128), HT, TPC]   per core
    #  w1/w3 [b, fc, 128p(h%128), HT(h//128), FC]
    #  w2  [b, fc, 128p(f%128), FT(f//128 within chunk), H]
    w1t = np.ascontiguousarray(
        w1g.reshape(nb, HT, 128, N_FC, FC).transpose(0, 3, 2, 1, 4)
    )
    w3t = np.ascontiguousarray(
        w3g.reshape(nb, HT, 128, N_FC, FC).transpose(0, 3, 2, 1, 4)
    )
    w2t = np.ascontiguousarray(
        w2g.reshape(nb, N_FC, FT, 128, H).transpose(0, 1, 3, 2, 4)
    )

    in_maps = []
    for c in range(N_CORES):
        sl = slice(c * nbpc, (c + 1) * nbpc)
        xc = xb[sl]  # [nbpc, TOK, H]
        # xT: [H, nbpc*TOK] -> [HT, 128, TPC] -> [128, HT, TPC]
        xt = np.ascontiguousarray(
            xc.reshape(nbpc * TOK, H).T.reshape(HT, 128, nbpc * TOK)
            .transpose(1, 0, 2)
        )
        in_maps.append({
            "xt": xt,
            "w1": np.ascontiguousarray(w1t[sl]),
            "w3": np.ascontiguousarray(w3t[sl]),
            "w2": np.ascontiguousarray(w2t[sl]),
        })
    return in_maps


def _run(hidden_states, w1, w3, w2, group_sizes, trace=False, **run_kwargs):
    from concourse.bass_utils import run_bass_kernel_spmd

    buckets = _plan_buckets(group_sizes)
    nbpc = -(-len(buckets) // N_CORES)  # ceil
    nb = nbpc * N_CORES
    while len(buckets) < nb:
        buckets.append((0, 0, 0))  # padding buckets (zero tokens)

    nc = _get_compiled(nbpc)
    in_maps = _prepare_in_maps(hidden_states, w1, w3, w2, buckets, nbpc)
    res = run_bass_kernel_spmd(
        nc, in_maps, core_ids=list(range(N_CORES)), trace=trace, **run_kwargs
    )

    out_buckets = np.concatenate(
        [r["out"].reshape(nbpc, TOK, H) for r in res.results], axis=0
    )  # [nb, TOK, H] float32

    T_total = int(np.asarray(group_sizes).sum())
    out = np.zeros((hidden_states.shape[0], H), dtype=np.float32)
    for i, (e, s, n) in enumerate(buckets):
        if n:
            out[s:s + n] = out_buckets[i, :n]
    del T_total
    return out, res


def kernel(hidden_states, w1, w3, w2, group_sizes):
    out, _ = _run(hidden_states, w1, w3, w2, group_sizes)
    return out
